# revision 13
# baseline (speedup 1.0000x reference)
"""Trainium2 Bass kernel for a pre-norm transformer encoder layer.

Problem: x(8,1024,1024) fp32; LN1 -> MHA(16 heads, hd=64) + residual;
LN2 -> FFN(4096, exact gelu) + residual.

Strategy:
- Data-parallel: one batch element per NeuronCore (8 cores, no collectives).
- All matmuls in float32r (reduced-precision fp32 matmul mode, 1 cycle/row
  at N=512, ~1.5e-4 median rel error vs ~2.3e-3 for bf16).
- LN gamma/beta folded into the following projection weights on the host;
  1/sqrt(hd) folded into Wq.
- Activations kept transposed (feature dim on partitions) where contractions
  need it: zT, Q^T, K^T, ctx^T, u^T. PE transposes (exact for fp32) produce zT.
- Attention computed as scores^T = K^T.T @ Q^T per head (softmax reduction
  along partitions is avoided by computing sums via a [V | 1] augmented
  AV matmul: the 65th output row is the softmax denominator).
- exp() without max-subtraction (scores ~ N(0,1), |s| < ~7, exact vs
  reference up to fp rounding since softmax is shift-invariant).
- Softmax normalization applied *after* the AV matmul on the small ctx
  tensor: 1/sums via DVE reciprocal, partition-broadcast on GPSIMD,
  one in-place DVE multiply per ctx^T tile.
- FFN split into two 512-token halves so the gelu(u^T) intermediate
  (8 MB fp32 per half) fits SBUF.
"""

import numpy as np
from contextlib import ExitStack

import concourse.bass as bass
import concourse.tile as tile
import concourse.mybir as mybir
from concourse import bacc
from concourse import bass_utils

F32 = mybir.dt.float32
F32R = mybir.dt.float32r
AF = mybir.ActivationFunctionType

S, D, H, HD, FF = 1024, 1024, 16, 64, 4096
ST, DT, FT = S // 128, D // 128, FF // 128
EPS = 1e-5
NCORES = 8

_CACHE = {}


def _build_program(with_bias, dbg=False):
    nc = bacc.Bacc("TRN2", target_bir_lowering=False, debug=False,
                   num_devices=NCORES)

    din = {}
    for name, shape in [
        ("x", (S, D)), ("wq", (D, D)), ("wk", (D, D)), ("wv", (D, D)),
        ("wo", (D, D)), ("w1", (D, FF)), ("w2", (FF, D)),
        ("bq", (1, D)), ("bk", (1, D)), ("bv", (1, D)), ("bo", (1, D)),
        ("b1", (1, FF)), ("b2", (1, D)),
        ("ident", (128, 128)), ("ones", (1, 512)), ("onescol", (1, 16)),
    ]:
        din[name] = nc.dram_tensor(name, shape, F32, kind="ExternalInput").ap()
    d_out = nc.dram_tensor("out", (S, D), F32, kind="ExternalOutput").ap()
    ddbg = {}
    if dbg:
        for name, shape in [("dbg_zT", (D, S)), ("dbg_qT", (D, S)),
                            ("dbg_kT", (D, S)), ("dbg_v65", (ST, 128, 1040)),
                            ("dbg_ctxT", (D, S)), ("dbg_x2", (S, D)),
                            ("dbg_z2T", (D, S)), ("dbg_rbc", (8, 128, 1024)),
                            ("dbg_sums", (8, 128, 1024))]:
            ddbg[name] = nc.dram_tensor(name, shape, F32, kind="ExternalOutput").ap()

    with tile.TileContext(nc) as tc, ExitStack() as ctx:
        _body(nc, tc, ctx, din, d_out, with_bias, ddbg)
    nc.compile()
    return nc


def _body(nc, tc, ctx, din, d_out, with_bias, ddbg=None):
    ddbg = ddbg or {}
    def dump(name, ap, dst):
        if name in ddbg:
            nc.sync.dma_start(dst, ap)
    mega = ctx.enter_context(tc.tile_pool(name="mega", bufs=32))
    expp = ctx.enter_context(tc.tile_pool(name="expp", bufs=4))
    wp = ctx.enter_context(tc.tile_pool(name="wp", bufs=4))
    w2p = ctx.enter_context(tc.tile_pool(name="w2p", bufs=4))
    outp = ctx.enter_context(tc.tile_pool(name="outp", bufs=3))
    smallp = ctx.enter_context(tc.tile_pool(name="smallp", bufs=4))
    cstp = ctx.enter_context(tc.tile_pool(name="cstp", bufs=1))
    attp = ctx.enter_context(tc.tile_pool(name="attp", bufs=1))
    biasp = ctx.enter_context(tc.tile_pool(name="biasp", bufs=4)) if with_bias else None

    MEGA = dict(tag="mega")

    # constants
    ident = cstp.tile([128, 128], F32, tag="ident")
    nc.sync.dma_start(ident[:], din["ident"])
    ones = cstp.tile([1, 512], F32R, tag="ones")
    nc.sync.dma_start(ones[:], din["ones"].bitcast(F32R))
    onescol = cstp.tile([1, 16], F32R, tag="onescol")
    nc.sync.dma_start(onescol[:], din["onescol"].bitcast(F32R))
    eps_t = cstp.tile([128, 1], F32, tag="eps")
    nc.vector.memset(eps_t[:], EPS)

    def bias_slice(dsrc, lo, n):
        """DMA a [1, n] f32r bias slice into a rotating tile."""
        bt = biasp.tile([1, 512], F32R, tag="brow")
        nc.sync.dma_start(bt[0:1, 0:n], dsrc[0:1, lo:lo + n].bitcast(F32R))
        return bt[0:1, 0:n]

    # ---------------- LayerNorm -> transposed z ----------------
    def layernorm_T(src_tiles, ps_pool, scope):
        """src_tiles: 8 [128,1024] f32 (tokens x d). Returns 8 [128,1024] f32r
        tiles of z^T (d x tokens), z = (x - mu) * rsqrt(var + eps)."""
        z_tiles = []
        for t in range(ST):
            xt = src_tiles[t]
            stats = smallp.tile([128, 2, 6], F32, tag="stats")
            nc.vector.bn_stats(stats[:, 0, :], xt[:, 0:512])
            nc.vector.bn_stats(stats[:, 1, :], xt[:, 512:1024])
            mv = smallp.tile([128, 2], F32, tag="mv")
            nc.vector.bn_aggr(mv[:], stats[:])
            std = smallp.tile([128, 1], F32, tag="std")
            nc.scalar.activation(std[:], mv[:, 1:2], AF.Sqrt, bias=eps_t[:])
            rstd = smallp.tile([128, 1], F32, tag="rstd")
            nc.vector.reciprocal(rstd[:], std[:])
            negmu = smallp.tile([128, 1], F32, tag="negmu")
            nc.vector.tensor_scalar_mul(negmu[:], mv[:, 0:1], -1.0)
            zt = mega.tile([128, 1024], F32, **MEGA)
            nc.vector.tensor_scalar(zt[:], xt[:], negmu[:], rstd[:],
                                    op0=mybir.AluOpType.add,
                                    op1=mybir.AluOpType.mult)
            z_tiles.append(zt)
        zT = []
        for j in range(DT):
            pt = ps_pool.tile([128, 1024], F32, tag="pst")
            for t in range(ST):
                nc.tensor.transpose(pt[:, t * 128:(t + 1) * 128],
                                    z_tiles[t][:, j * 128:(j + 1) * 128],
                                    ident[:])
            zTj = mega.tile([128, 1024], F32R, **MEGA)
            nc.vector.tensor_copy(zTj[:], pt[:])
            zT.append(zTj)
        return zT

    # ---------------- Phase 1: LN1 ----------------
    with tc.tile_pool(name="ps_ln1", bufs=2, space="PSUM") as ps_ln1:
        x_tiles = []
        for t in range(ST):
            xt = mega.tile([128, 1024], F32, **MEGA)
            nc.sync.dma_start(xt[:], din["x"][t * 128:(t + 1) * 128, :])
            x_tiles.append(xt)
        zT = layernorm_T(x_tiles, ps_ln1, "ln1")
        if "dbg_zT" in ddbg:
            for j in range(DT):
                nc.sync.dma_start(ddbg["dbg_zT"][j * 128:(j + 1) * 128, :], zT[j][:].bitcast(F32))

    # ---------------- Phase 2: QKV ----------------
    with tc.tile_pool(name="ps_qkv", bufs=4, space="PSUM") as ps_qkv:
        def proj_T(dw, dbias):
            """out[o] tiles: (z @ W)^T as [dq x s], o = dq tile."""
            res = []
            for o in range(DT):
                wcol = wp.tile([128, 1024], F32R, tag="w")
                src = din[dw][:, o * 128:(o + 1) * 128]
                src = src.rearrange("(dt p) m -> p dt m", p=128)
                dst = wcol[:].rearrange("p (dt m) -> p dt m", m=128)
                nc.sync.dma_start(dst, src.bitcast(F32R))
                p = ps_qkv.tile([128, 1024], F32, tag="psqkv")
                bs = bias_slice(dbias, o * 128, 128) if with_bias else None
                for c in range(2):
                    if with_bias:
                        nc.tensor.matmul(p[:, c * 512:(c + 1) * 512], bs,
                                         ones[0:1, 0:512], start=True, stop=False)
                    for d in range(DT):
                        nc.tensor.matmul(
                            p[:, c * 512:(c + 1) * 512],
                            wcol[:, d * 128:(d + 1) * 128],
                            zT[d][:, c * 512:(c + 1) * 512],
                            start=(d == 0 and not with_bias), stop=(d == DT - 1))
                ot = mega.tile([128, 1024], F32R, **MEGA)
                nc.scalar.copy(ot[:], p[:])
                res.append(ot)
            return res

        QT = proj_T("wq", "bq")
        KT = proj_T("wk", "bk")
        if "dbg_qT" in ddbg:
            for j in range(DT):
                nc.sync.dma_start(ddbg["dbg_qT"][j * 128:(j + 1) * 128, :], QT[j][:].bitcast(F32))
                nc.sync.dma_start(ddbg["dbg_kT"][j * 128:(j + 1) * 128, :], KT[j][:].bitcast(F32))

        # V natural [tokens x dv], stored 65-strided with a ones column per head
        V65 = [mega.tile([128, 1040], F32R, name=f"v65_{i}", **MEGA) for i in range(ST)]
        for th in range(2):
            ptiles = [ps_qkv.tile([128, 1024], F32, name=f"psv_{th}_{i}", tag="psqkv") for i in range(4)]
            if with_bias:
                for tl in range(4):
                    for c in range(2):
                        bs = bias_slice(din["bv"], c * 512, 512)
                        nc.tensor.matmul(ptiles[tl][:, c * 512:(c + 1) * 512],
                                         ones[0:1, 0:128], bs,
                                         start=True, stop=False)
            for d in range(DT):
                wrow = wp.tile([128, 1024], F32R, tag="w")
                nc.sync.dma_start(wrow[:], din["wv"][d * 128:(d + 1) * 128, :].bitcast(F32R))
                for tl in range(4):
                    t = th * 4 + tl
                    for c in range(2):
                        nc.tensor.matmul(
                            ptiles[tl][:, c * 512:(c + 1) * 512],
                            zT[d][:, t * 128:(t + 1) * 128],
                            wrow[:, c * 512:(c + 1) * 512],
                            start=(d == 0 and not with_bias), stop=(d == DT - 1))
            for tl in range(4):
                t = th * 4 + tl
                pv = ptiles[tl][:].rearrange("p (h c) -> p h c", c=64)
                dv = V65[t][:].rearrange("p (h c) -> p h c", c=65)[:, :, 0:64]
                nc.vector.tensor_copy(dv, pv)
                oc = V65[t][:].rearrange("p (h c) -> p h c", c=65)[:, :, 64:65]
                nc.gpsimd.partition_broadcast(
                    oc, onescol[:].rearrange("p (h c) -> p h c", c=1))

    if "dbg_v65" in ddbg:
        for t in range(ST):
            nc.sync.dma_start(ddbg["dbg_v65"][t], V65[t][:].bitcast(F32))

    # ---------------- Phase 3: attention ----------------
    ctxT = [mega.tile([128, 1024], F32R, name=f"ctxT_{i}", **MEGA) for i in range(DT)]
    with tc.tile_pool(name="ps_s", bufs=2, space="PSUM") as ps_s, \
         tc.tile_pool(name="ps_av", bufs=2, space="PSUM") as ps_av:
        for hp in range(8):
            pavA = ps_av.tile([128, 1024], F32, tag="psav")
            pavB = ps_av.tile([128, 1024], F32, tag="psav")
            for kt in range(ST):
                for c in range(2):
                    sc = ps_s.tile([128, 1024], F32, tag="pss")
                    nc.tensor.matmul(sc[:, 0:512],
                                     KT[hp][0:64, kt * 128:(kt + 1) * 128],
                                     QT[hp][0:64, c * 512:(c + 1) * 512],
                                     start=True, stop=True)
                    nc.tensor.matmul(sc[:, 512:1024],
                                     KT[hp][64:128, kt * 128:(kt + 1) * 128],
                                     QT[hp][64:128, c * 512:(c + 1) * 512],
                                     start=True, stop=True)
                    e = expp.tile([128, 1024], F32R, tag="exp")
                    nc.scalar.activation(e[:], sc[:], AF.Exp)
                    nc.tensor.matmul(pavA[0:65, c * 512:(c + 1) * 512],
                                     V65[kt][:, (2 * hp) * 65:(2 * hp) * 65 + 65],
                                     e[:, 0:512],
                                     start=(kt == 0), stop=(kt == ST - 1))
                    nc.tensor.matmul(pavB[0:65, c * 512:(c + 1) * 512],
                                     V65[kt][:, (2 * hp + 1) * 65:(2 * hp + 1) * 65 + 65],
                                     e[:, 512:1024],
                                     start=(kt == 0), stop=(kt == ST - 1))
            # softmax denominators for the pair: rows 0 (head A) and 32 (head B)
            psum_pair = attp.tile([128, 1024], F32, tag="psum_pair")
            nc.scalar.copy(psum_pair[0:1, :], pavA[64:65, :])
            nc.scalar.copy(psum_pair[32:33, :], pavB[64:65, :])
            nc.vector.tensor_copy(ctxT[hp][0:64, :], pavA[0:64, :])
            nc.vector.tensor_copy(ctxT[hp][64:128, :], pavB[0:64, :])
            # 1/sums (~2 ULP); garbage rows besides 0/32 are never read
            prec = attp.tile([128, 1024], F32, tag="prec")
            pscr = attp.tile([128, 1024], F32, tag="pscr")
            nc.vector.reciprocal_approx_accurate(prec[:], psum_pair[:], pscr[:])
            rbc = attp.tile([128, 1024], F32, tag="rbc")
            bmask = [0] * 32
            nc.vector.stream_shuffle(rbc[0:32, :], prec[0:32, :], bmask)
            nc.vector.stream_shuffle(rbc[32:64, :], prec[0:32, :], bmask)
            nc.vector.stream_shuffle(rbc[64:96, :], prec[32:64, :], bmask)
            nc.vector.stream_shuffle(rbc[96:128, :], prec[32:64, :], bmask)
            if "dbg_rbc" in ddbg:
                nc.sync.dma_start(ddbg["dbg_rbc"][hp], rbc[:])
                nc.sync.dma_start(ddbg["dbg_sums"][hp], psum_pair[:])
            nc.vector.tensor_mul(ctxT[hp][:], ctxT[hp][:].bitcast(F32), rbc[:])

    if "dbg_ctxT" in ddbg:
        for j in range(DT):
            nc.sync.dma_start(ddbg["dbg_ctxT"][j * 128:(j + 1) * 128, :], ctxT[j][:].bitcast(F32))

    # ---------------- Phase 4: out-proj + residual ----------------
    x2_tiles = [None] * ST
    with tc.tile_pool(name="ps_o", bufs=4, space="PSUM") as ps_o:
        for th in range(2):
            ptiles = [ps_o.tile([128, 1024], F32, name=f"pso_{th}_{i}", tag="pso") for i in range(4)]
            if with_bias:
                for tl in range(4):
                    for c in range(2):
                        bs = bias_slice(din["bo"], c * 512, 512)
                        nc.tensor.matmul(ptiles[tl][:, c * 512:(c + 1) * 512],
                                         ones[0:1, 0:128], bs,
                                         start=True, stop=False)
            for d in range(DT):
                wrow = wp.tile([128, 1024], F32R, tag="w")
                nc.sync.dma_start(wrow[:], din["wo"][d * 128:(d + 1) * 128, :].bitcast(F32R))
                for tl in range(4):
                    t = th * 4 + tl
                    for c in range(2):
                        nc.tensor.matmul(
                            ptiles[tl][:, c * 512:(c + 1) * 512],
                            ctxT[d][:, t * 128:(t + 1) * 128],
                            wrow[:, c * 512:(c + 1) * 512],
                            start=(d == 0 and not with_bias), stop=(d == DT - 1))
            for tl in range(4):
                t = th * 4 + tl
                xres = mega.tile([128, 1024], F32, **MEGA)
                nc.sync.dma_start(xres[:], din["x"][t * 128:(t + 1) * 128, :])
                x2t = mega.tile([128, 1024], F32, **MEGA)
                nc.vector.tensor_add(x2t[:], xres[:], ptiles[tl][:])
                x2_tiles[t] = x2t

    if "dbg_x2" in ddbg:
        for t in range(ST):
            nc.sync.dma_start(ddbg["dbg_x2"][t * 128:(t + 1) * 128, :], x2_tiles[t][:])

    # ---------------- Phase 5: LN2 ----------------
    with tc.tile_pool(name="ps_ln2", bufs=2, space="PSUM") as ps_ln2:
        z2T = layernorm_T(x2_tiles, ps_ln2, "ln2")
    if "dbg_z2T" in ddbg:
        for j in range(DT):
            nc.sync.dma_start(ddbg["dbg_z2T"][j * 128:(j + 1) * 128, :], z2T[j][:].bitcast(F32))

    # ---------------- Phase 6: FFN + residual ----------------
    with tc.tile_pool(name="ps_f1", bufs=3, space="PSUM") as ps_f1, \
         tc.tile_pool(name="ps_f2", bufs=4, space="PSUM") as ps_f2:
        for hf in range(2):
            gu_tiles = []
            for fp in range(FT // 2):
                gt = mega.tile([128, 1024], F32R, **MEGA)
                for sub in range(2):
                    ft = fp * 2 + sub
                    wcol = wp.tile([128, 1024], F32R, tag="w")
                    src = din["w1"][:, ft * 128:(ft + 1) * 128]
                    src = src.rearrange("(dt p) m -> p dt m", p=128)
                    dst = wcol[:].rearrange("p (dt m) -> p dt m", m=128)
                    nc.sync.dma_start(dst, src.bitcast(F32R))
                    p = ps_f1.tile([128, 512], F32, tag="psf1")
                    if with_bias:
                        bs = bias_slice(din["b1"], ft * 128, 128)
                        nc.tensor.matmul(p[:], bs, ones[0:1, 0:512],
                                         start=True, stop=False)
                    for d in range(DT):
                        nc.tensor.matmul(
                            p[:],
                            wcol[:, d * 128:(d + 1) * 128],
                            z2T[d][:, hf * 512:(hf + 1) * 512],
                            start=(d == 0 and not with_bias), stop=(d == DT - 1))
                    nc.scalar.activation(gt[:, sub * 512:(sub + 1) * 512], p[:],
                                         AF.Gelu)
                gu_tiles.append(gt)
            for c in range(2):
                accs = [ps_f2.tile([128, 512], F32, name=f"acc_{hf}_{c}_{i}", tag="psf2") for i in range(4)]
                if with_bias:
                    for tl in range(4):
                        bs = bias_slice(din["b2"], c * 512, 512)
                        nc.tensor.matmul(accs[tl][:], ones[0:1, 0:128], bs,
                                         start=True, stop=False)
                for ft in range(FT):
                    w2t = w2p.tile([128, 512], F32R, tag="w2")
                    nc.sync.dma_start(
                        w2t[:],
                        din["w2"][ft * 128:(ft + 1) * 128,
                                  c * 512:(c + 1) * 512].bitcast(F32R))
                    for tl in range(4):
                        lo = (ft % 2) * 512 + tl * 128
                        nc.tensor.matmul(
                            accs[tl][:],
                            gu_tiles[ft // 2][:, lo:lo + 128],
                            w2t[:],
                            start=(ft == 0 and not with_bias), stop=(ft == FT - 1))
                for tl in range(4):
                    t = hf * 4 + tl
                    ot = outp.tile([128, 512], F32, tag="outp")
                    nc.vector.tensor_add(ot[:],
                                         x2_tiles[t][:, c * 512:(c + 1) * 512],
                                         accs[tl][:])
                    nc.sync.dma_start(
                        d_out[t * 128:(t + 1) * 128, c * 512:(c + 1) * 512],
                        ot[:])


def _get_program(with_bias):
    key = ("prog", with_bias)
    if key not in _CACHE:
        _CACHE[key] = _build_program(with_bias)
    return _CACHE[key]


def _prepare(x, Wq, bq, Wk, bk, Wv, bv, Wo, bo, W1, b1, W2, b2,
             g1, be1, g2, be2):
    x = np.asarray(x, dtype=np.float32)
    f64 = np.float64

    # Fold LN affine params into the following projections (exact algebra):
    # (z*g + be) @ W + b = z @ (g[:,None]*W) + (be @ W + b).
    # Also fold the 1/sqrt(hd) attention scale into Wq/bq.
    scale_q = 1.0 / np.sqrt(np.float64(HD))
    wq_eff = (np.asarray(g1, f64)[:, None] * np.asarray(Wq, f64)) * scale_q
    bq_eff = (np.asarray(be1, f64) @ np.asarray(Wq, f64) + np.asarray(bq, f64)) * scale_q
    wk_eff = np.asarray(g1, f64)[:, None] * np.asarray(Wk, f64)
    bk_eff = np.asarray(be1, f64) @ np.asarray(Wk, f64) + np.asarray(bk, f64)
    wv_eff = np.asarray(g1, f64)[:, None] * np.asarray(Wv, f64)
    bv_eff = np.asarray(be1, f64) @ np.asarray(Wv, f64) + np.asarray(bv, f64)
    w1_eff = np.asarray(g2, f64)[:, None] * np.asarray(W1, f64)
    b1_eff = np.asarray(be2, f64) @ np.asarray(W1, f64) + np.asarray(b1, f64)

    biases = [bq_eff, bk_eff, bv_eff, np.asarray(bo, f64),
              b1_eff, np.asarray(b2, f64)]
    with_bias = any(np.any(b != 0.0) for b in biases)

    nc = _get_program(with_bias)

    common = {
        "wq": np.ascontiguousarray(wq_eff, dtype=np.float32),
        "wk": np.ascontiguousarray(wk_eff, dtype=np.float32),
        "wv": np.ascontiguousarray(wv_eff, dtype=np.float32),
        "wo": np.ascontiguousarray(Wo, dtype=np.float32),
        "w1": np.ascontiguousarray(w1_eff, dtype=np.float32),
        "w2": np.ascontiguousarray(W2, dtype=np.float32),
        "bq": bq_eff.astype(np.float32).reshape(1, D),
        "bk": bk_eff.astype(np.float32).reshape(1, D),
        "bv": bv_eff.astype(np.float32).reshape(1, D),
        "bo": np.asarray(bo, np.float32).reshape(1, D),
        "b1": b1_eff.astype(np.float32).reshape(1, FF),
        "b2": np.asarray(b2, np.float32).reshape(1, D),
        "ident": np.eye(128, dtype=np.float32),
        "ones": np.ones((1, 512), dtype=np.float32),
        "onescol": np.ones((1, 16), dtype=np.float32),
    }
    in_maps = []
    for b in range(NCORES):
        m = dict(common)
        m["x"] = np.ascontiguousarray(x[b])
        in_maps.append(m)
    return nc, in_maps


def kernel(**inputs):
    nc, in_maps = _prepare(**inputs)
    res = bass_utils.run_bass_kernel_spmd(nc, in_maps,
                                          core_ids=list(range(NCORES)))
    out = np.stack([res.results[b]["out"] for b in range(NCORES)], axis=0)
    return out.astype(np.float32)


def _timed_run(inputs):
    """Test-harness helper: rerun with NTFF tracing to get HW exec time."""
    nc, in_maps = _prepare(**inputs)
    try:
        return bass_utils.run_bass_kernel_spmd(
            nc, in_maps, core_ids=list(range(NCORES)), trace=True)
    except Exception as e:
        print(f"traced run failed: {e}")
        return None


# revision 14
# speedup vs baseline: 1.2104x; 1.2104x over previous
"""Trainium2 Bass kernel for a pre-norm transformer encoder layer.

Problem: x(8,1024,1024) fp32; LN1 -> MHA(16 heads, hd=64) + residual;
LN2 -> FFN(4096, exact gelu) + residual.

Strategy:
- Data-parallel: one batch element per NeuronCore (8 cores, no collectives).
- Matmuls in bf16 (1 cycle/row on the PE; fp32r fallback available).
  PSUM accumulation, LN statistics, softmax denominators and both residual
  adds stay fp32.
- LN gamma/beta folded into the following projection weights on the host;
  1/sqrt(hd) folded into Wq.
- Activations kept transposed (feature dim on partitions) where contractions
  need them: zT, Q^T, K^T, ctx^T, u^T. PE transposes produce zT.
- Attention computed as scores^T = K^T.T @ Q^T per head pair (two heads run
  concurrently in distinct PE row groups); softmax-over-partitions is avoided
  by a [V | 1] augmented AV matmul whose 65th output row is the denominator.
- exp() without max-subtraction (scores ~ N(0,1), exact since softmax is
  shift-invariant and values stay far from fp32 overflow).
- Softmax normalization applied after the AV matmul on the small ctx tensor:
  1/sums via DVE approx reciprocal (~2 ULP), partition-broadcast via
  stream_shuffle, one in-place DVE multiply per ctx^T tile.
- FFN split into two 512-token halves so the gelu(u^T) intermediate fits SBUF.
"""

import numpy as np
import ml_dtypes
from contextlib import ExitStack

import concourse.bass as bass
import concourse.tile as tile
import concourse.mybir as mybir
from concourse import bacc
from concourse import bass_utils

F32 = mybir.dt.float32
F32R = mybir.dt.float32r
BF16 = mybir.dt.bfloat16
AF = mybir.ActivationFunctionType

S, D, H, HD, FF = 1024, 1024, 16, 64, 4096
ST, DT, FT = S // 128, D // 128, FF // 128
EPS = 1e-5
NCORES = 8
MM_MODE = "bf16"          # "bf16" | "f32r"

_CACHE = {}


def _np_dt(mode):
    return ml_dtypes.bfloat16 if mode == "bf16" else np.float32


def _build_program(with_bias, mode=MM_MODE, dbg=False):
    MD = BF16 if mode == "bf16" else F32R
    nc = bacc.Bacc("TRN2", target_bir_lowering=False, debug=False,
                   num_devices=NCORES)

    din = {}
    for name, shape, dt in [
        ("x", (S, D), F32),
        ("wq", (D, D), MD), ("wk", (D, D), MD), ("wv", (D, D), MD),
        ("wo", (D, D), MD), ("w1", (D, FF), MD), ("w2", (FF, D), MD),
        ("bq", (1, D), MD), ("bk", (1, D), MD), ("bv", (1, D), MD),
        ("bo", (1, D), MD), ("b1", (1, FF), MD), ("b2", (1, D), MD),
        ("ident", (128, 128), MD if mode == "bf16" else F32),
        ("ones", (1, 512), MD), ("onescol", (1, 16), MD),
    ]:
        din[name] = nc.dram_tensor(name, shape, dt, kind="ExternalInput").ap()
    d_out = nc.dram_tensor("out", (S, D), F32, kind="ExternalOutput").ap()
    ddbg = {}
    if dbg:
        for name, shape, dt in [
            ("dbg_zT", (D, S), MD), ("dbg_qT", (D, S), MD),
            ("dbg_kT", (D, S), MD), ("dbg_v65", (ST, 128, 1040), MD),
            ("dbg_ctxT", (D, S), MD), ("dbg_x2", (S, D), F32),
            ("dbg_z2T", (D, S), MD), ("dbg_rbc", (8, 128, 1024), F32),
            ("dbg_sums", (8, 128, 1024), F32),
        ]:
            ddbg[name] = nc.dram_tensor(name, shape, dt, kind="ExternalOutput").ap()

    with tile.TileContext(nc) as tc, ExitStack() as ctx:
        _body(nc, tc, ctx, din, d_out, with_bias, mode, ddbg)
    nc.compile()
    return nc


def _body(nc, tc, ctx, din, d_out, with_bias, mode, ddbg=None):
    ddbg = ddbg or {}
    MD = BF16 if mode == "bf16" else F32R
    # z (transpose input) and the transpose PSUM must share a dtype the PE
    # transpose supports exactly: bf16 in bf16 mode, fp32 in f32r mode
    # (f32r transpose is not exact; copies cast psum fp32 -> f32r after).
    ZD = BF16 if mode == "bf16" else F32
    mf32 = ctx.enter_context(tc.tile_pool(name="mf32", bufs=14))
    mbf = ctx.enter_context(tc.tile_pool(name="mbf", bufs=34))
    expp = ctx.enter_context(tc.tile_pool(name="expp", bufs=6))
    wp = ctx.enter_context(tc.tile_pool(name="wp", bufs=4))
    w2p = ctx.enter_context(tc.tile_pool(name="w2p", bufs=4))
    outp = ctx.enter_context(tc.tile_pool(name="outp", bufs=3))
    smallp = ctx.enter_context(tc.tile_pool(name="smallp", bufs=4))
    cstp = ctx.enter_context(tc.tile_pool(name="cstp", bufs=1))
    attp = ctx.enter_context(tc.tile_pool(name="attp", bufs=2))
    biasp = ctx.enter_context(tc.tile_pool(name="biasp", bufs=4)) if with_bias else None

    # constants
    ident = cstp.tile([128, 128], din["ident"].dtype, tag="ident")
    nc.sync.dma_start(ident[:], din["ident"])
    ones = cstp.tile([1, 512], MD, tag="ones")
    nc.sync.dma_start(ones[:], din["ones"])
    onescol = cstp.tile([1, 16], MD, tag="onescol")
    nc.sync.dma_start(onescol[:], din["onescol"])
    eps_t = cstp.tile([128, 1], F32, tag="eps")
    nc.vector.memset(eps_t[:], EPS)

    def bias_slice(dsrc, lo, n):
        bt = biasp.tile([1, 512], MD, tag="brow")
        nc.sync.dma_start(bt[0:1, 0:n], dsrc[0:1, lo:lo + n])
        return bt[0:1, 0:n]

    # ---------------- LayerNorm -> transposed z ----------------
    def layernorm_T(src_tiles, ps_pool):
        """src_tiles: 8 [128,1024] f32 (tokens x d). Returns 8 [128,1024] MD
        tiles of z^T (d x tokens), z = (x - mu) * rsqrt(var + eps)."""
        z_tiles = []
        for t in range(ST):
            xt = src_tiles[t]
            stats = smallp.tile([128, 2, 6], F32, tag="stats")
            nc.vector.bn_stats(stats[:, 0, :], xt[:, 0:512])
            nc.vector.bn_stats(stats[:, 1, :], xt[:, 512:1024])
            mv = smallp.tile([128, 2], F32, tag="mv")
            nc.vector.bn_aggr(mv[:], stats[:])
            std = smallp.tile([128, 1], F32, tag="std")
            nc.scalar.activation(std[:], mv[:, 1:2], AF.Sqrt, bias=eps_t[:])
            rstd = smallp.tile([128, 1], F32, tag="rstd")
            nc.vector.reciprocal(rstd[:], std[:])
            negmu = smallp.tile([128, 1], F32, tag="negmu")
            nc.vector.tensor_scalar_mul(negmu[:], mv[:, 0:1], -1.0)
            zt = mbf.tile([128, 1024], ZD, tag="mbf")
            nc.vector.tensor_scalar(zt[:], xt[:], negmu[:], rstd[:],
                                    op0=mybir.AluOpType.add,
                                    op1=mybir.AluOpType.mult)
            z_tiles.append(zt)
        zT = []
        for j in range(DT):
            pt = ps_pool.tile([128, 1024], ZD, tag="pst")
            for t in range(ST):
                nc.tensor.transpose(pt[:, t * 128:(t + 1) * 128],
                                    z_tiles[t][:, j * 128:(j + 1) * 128],
                                    ident[:])
            zTj = mbf.tile([128, 1024], MD, tag="mbf")
            nc.vector.tensor_copy(zTj[:], pt[:])
            zT.append(zTj)
        return zT

    # ---------------- Phase 1: LN1 ----------------
    with tc.tile_pool(name="ps_ln1", bufs=2, space="PSUM") as ps_ln1:
        x_tiles = []
        for t in range(ST):
            xt = mf32.tile([128, 1024], F32, tag="mf32")
            nc.sync.dma_start(xt[:], din["x"][t * 128:(t + 1) * 128, :])
            x_tiles.append(xt)
        zT = layernorm_T(x_tiles, ps_ln1)
        if "dbg_zT" in ddbg:
            for j in range(DT):
                nc.sync.dma_start(ddbg["dbg_zT"][j * 128:(j + 1) * 128, :], zT[j][:])

    # ---------------- Phase 2: QKV ----------------
    with tc.tile_pool(name="ps_qkv", bufs=4, space="PSUM") as ps_qkv:
        def proj_T(dw, dbias):
            """(z @ W)^T tiles [dq x s]."""
            res = []
            for o in range(DT):
                wcol = wp.tile([128, 1024], MD, tag="w")
                src = din[dw][:, o * 128:(o + 1) * 128]
                src = src.rearrange("(dt p) m -> p dt m", p=128)
                dst = wcol[:].rearrange("p (dt m) -> p dt m", m=128)
                nc.sync.dma_start(dst, src)
                p = ps_qkv.tile([128, 1024], F32, tag="psqkv")
                bs = bias_slice(din[dbias], o * 128, 128) if with_bias else None
                for c in range(2):
                    if with_bias:
                        nc.tensor.matmul(p[:, c * 512:(c + 1) * 512], bs,
                                         ones[0:1, 0:512], start=True, stop=False)
                    for d in range(DT):
                        nc.tensor.matmul(
                            p[:, c * 512:(c + 1) * 512],
                            wcol[:, d * 128:(d + 1) * 128],
                            zT[d][:, c * 512:(c + 1) * 512],
                            start=(d == 0 and not with_bias), stop=(d == DT - 1))
                ot = mbf.tile([128, 1024], MD, tag="mbf")
                nc.scalar.copy(ot[:], p[:])
                res.append(ot)
            return res

        QT = proj_T("wq", "bq")
        KT = proj_T("wk", "bk")
        if "dbg_qT" in ddbg:
            for j in range(DT):
                nc.sync.dma_start(ddbg["dbg_qT"][j * 128:(j + 1) * 128, :], QT[j][:])
                nc.sync.dma_start(ddbg["dbg_kT"][j * 128:(j + 1) * 128, :], KT[j][:])

        # V natural [tokens x dv], 65-strided with a ones column per head
        V65 = [mbf.tile([128, 1040], MD, name=f"v65_{i}", tag="mbf") for i in range(ST)]
        for th in range(2):
            ptiles = [ps_qkv.tile([128, 1024], F32, name=f"psv_{th}_{i}", tag="psqkv") for i in range(4)]
            if with_bias:
                for tl in range(4):
                    for c in range(2):
                        bs = bias_slice(din["bv"], c * 512, 512)
                        nc.tensor.matmul(ptiles[tl][:, c * 512:(c + 1) * 512],
                                         ones[0:1, 0:128], bs,
                                         start=True, stop=False)
            for d in range(DT):
                wrow = wp.tile([128, 1024], MD, tag="w")
                nc.sync.dma_start(wrow[:], din["wv"][d * 128:(d + 1) * 128, :])
                for tl in range(4):
                    t = th * 4 + tl
                    for c in range(2):
                        nc.tensor.matmul(
                            ptiles[tl][:, c * 512:(c + 1) * 512],
                            zT[d][:, t * 128:(t + 1) * 128],
                            wrow[:, c * 512:(c + 1) * 512],
                            start=(d == 0 and not with_bias), stop=(d == DT - 1))
            for tl in range(4):
                t = th * 4 + tl
                pv = ptiles[tl][:].rearrange("p (h c) -> p h c", c=64)
                dv = V65[t][:].rearrange("p (h c) -> p h c", c=65)[:, :, 0:64]
                nc.vector.tensor_copy(dv, pv)
                oc = V65[t][:].rearrange("p (h c) -> p h c", c=65)[:, :, 64:65]
                nc.gpsimd.partition_broadcast(
                    oc, onescol[:].rearrange("p (h c) -> p h c", c=1))
    if "dbg_v65" in ddbg:
        for t in range(ST):
            nc.sync.dma_start(ddbg["dbg_v65"][t], V65[t][:])

    # ---------------- Phase 3: attention ----------------
    ctxT = [mbf.tile([128, 1024], MD, name=f"ctxT_{i}", tag="mbf") for i in range(DT)]
    with tc.tile_pool(name="ps_s", bufs=2, space="PSUM") as ps_s, \
         tc.tile_pool(name="ps_av", bufs=2, space="PSUM") as ps_av:
        for hp in range(8):
            pavA = ps_av.tile([128, 1024], F32, tag="psav")
            pavB = ps_av.tile([128, 1024], F32, tag="psav")
            for kt in range(ST):
                for c in range(2):
                    sc = ps_s.tile([128, 1024], F32, tag="pss")
                    nc.tensor.matmul(sc[:, 0:512],
                                     KT[hp][0:64, kt * 128:(kt + 1) * 128],
                                     QT[hp][0:64, c * 512:(c + 1) * 512],
                                     start=True, stop=True)
                    nc.tensor.matmul(sc[:, 512:1024],
                                     KT[hp][64:128, kt * 128:(kt + 1) * 128],
                                     QT[hp][64:128, c * 512:(c + 1) * 512],
                                     start=True, stop=True)
                    e = expp.tile([128, 1024], MD, tag="exp")
                    nc.scalar.activation(e[:], sc[:], AF.Exp)
                    nc.tensor.matmul(pavA[0:65, c * 512:(c + 1) * 512],
                                     V65[kt][:, (2 * hp) * 65:(2 * hp) * 65 + 65],
                                     e[:, 0:512],
                                     start=(kt == 0), stop=(kt == ST - 1))
                    nc.tensor.matmul(pavB[0:65, c * 512:(c + 1) * 512],
                                     V65[kt][:, (2 * hp + 1) * 65:(2 * hp + 1) * 65 + 65],
                                     e[:, 512:1024],
                                     start=(kt == 0), stop=(kt == ST - 1))
            # denominators for the pair at rows 0 (head A) / 32 (head B)
            psum_pair = attp.tile([128, 1024], F32, tag="psum_pair")
            nc.scalar.copy(psum_pair[0:1, :], pavA[64:65, :])
            nc.scalar.copy(psum_pair[32:33, :], pavB[64:65, :])
            nc.vector.tensor_copy(ctxT[hp][0:64, :], pavA[0:64, :])
            nc.vector.tensor_copy(ctxT[hp][64:128, :], pavB[0:64, :])
            prec = attp.tile([128, 1024], F32, tag="prec")
            pscr = attp.tile([128, 1024], F32, tag="pscr")
            nc.vector.reciprocal_approx_accurate(prec[:], psum_pair[:], pscr[:])
            rbc = attp.tile([128, 1024], F32, tag="rbc")
            bmask = [0] * 32
            nc.vector.stream_shuffle(rbc[0:32, :], prec[0:32, :], bmask)
            nc.vector.stream_shuffle(rbc[32:64, :], prec[0:32, :], bmask)
            nc.vector.stream_shuffle(rbc[64:96, :], prec[32:64, :], bmask)
            nc.vector.stream_shuffle(rbc[96:128, :], prec[32:64, :], bmask)
            if "dbg_rbc" in ddbg:
                nc.sync.dma_start(ddbg["dbg_rbc"][hp], rbc[:])
                nc.sync.dma_start(ddbg["dbg_sums"][hp], psum_pair[:])
            nc.vector.tensor_mul(ctxT[hp][:], ctxT[hp][:], rbc[:])
    if "dbg_ctxT" in ddbg:
        for j in range(DT):
            nc.sync.dma_start(ddbg["dbg_ctxT"][j * 128:(j + 1) * 128, :], ctxT[j][:])

    # ---------------- Phase 4: out-proj + residual ----------------
    x2_tiles = [None] * ST
    with tc.tile_pool(name="ps_o", bufs=4, space="PSUM") as ps_o:
        for th in range(2):
            ptiles = [ps_o.tile([128, 1024], F32, name=f"pso_{th}_{i}", tag="pso") for i in range(4)]
            if with_bias:
                for tl in range(4):
                    for c in range(2):
                        bs = bias_slice(din["bo"], c * 512, 512)
                        nc.tensor.matmul(ptiles[tl][:, c * 512:(c + 1) * 512],
                                         ones[0:1, 0:128], bs,
                                         start=True, stop=False)
            for d in range(DT):
                wrow = wp.tile([128, 1024], MD, tag="w")
                nc.sync.dma_start(wrow[:], din["wo"][d * 128:(d + 1) * 128, :])
                for tl in range(4):
                    t = th * 4 + tl
                    for c in range(2):
                        nc.tensor.matmul(
                            ptiles[tl][:, c * 512:(c + 1) * 512],
                            ctxT[d][:, t * 128:(t + 1) * 128],
                            wrow[:, c * 512:(c + 1) * 512],
                            start=(d == 0 and not with_bias), stop=(d == DT - 1))
            for tl in range(4):
                t = th * 4 + tl
                xres = mf32.tile([128, 1024], F32, tag="mf32")
                nc.sync.dma_start(xres[:], din["x"][t * 128:(t + 1) * 128, :])
                x2t = mf32.tile([128, 1024], F32, tag="mf32")
                nc.vector.tensor_add(x2t[:], xres[:], ptiles[tl][:])
                x2_tiles[t] = x2t
    if "dbg_x2" in ddbg:
        for t in range(ST):
            nc.sync.dma_start(ddbg["dbg_x2"][t * 128:(t + 1) * 128, :], x2_tiles[t][:])

    # ---------------- Phase 5: LN2 ----------------
    with tc.tile_pool(name="ps_ln2", bufs=2, space="PSUM") as ps_ln2:
        z2T = layernorm_T(x2_tiles, ps_ln2)
    if "dbg_z2T" in ddbg:
        for j in range(DT):
            nc.sync.dma_start(ddbg["dbg_z2T"][j * 128:(j + 1) * 128, :], z2T[j][:])

    # ---------------- Phase 6: FFN + residual ----------------
    with tc.tile_pool(name="ps_f1", bufs=3, space="PSUM") as ps_f1, \
         tc.tile_pool(name="ps_f2", bufs=4, space="PSUM") as ps_f2:
        for hf in range(2):
            gu_tiles = []
            for fp in range(FT // 2):
                gt = mbf.tile([128, 1024], MD, tag="mbf")
                for sub in range(2):
                    ft = fp * 2 + sub
                    wcol = wp.tile([128, 1024], MD, tag="w")
                    src = din["w1"][:, ft * 128:(ft + 1) * 128]
                    src = src.rearrange("(dt p) m -> p dt m", p=128)
                    dst = wcol[:].rearrange("p (dt m) -> p dt m", m=128)
                    nc.sync.dma_start(dst, src)
                    p = ps_f1.tile([128, 512], F32, tag="psf1")
                    if with_bias:
                        bs = bias_slice(din["b1"], ft * 128, 128)
                        nc.tensor.matmul(p[:], bs, ones[0:1, 0:512],
                                         start=True, stop=False)
                    for d in range(DT):
                        nc.tensor.matmul(
                            p[:],
                            wcol[:, d * 128:(d + 1) * 128],
                            z2T[d][:, hf * 512:(hf + 1) * 512],
                            start=(d == 0 and not with_bias), stop=(d == DT - 1))
                    nc.scalar.activation(gt[:, sub * 512:(sub + 1) * 512], p[:],
                                         AF.Gelu)
                gu_tiles.append(gt)
            for c in range(2):
                accs = [ps_f2.tile([128, 512], F32, name=f"acc_{hf}_{c}_{i}", tag="psf2") for i in range(4)]
                if with_bias:
                    for tl in range(4):
                        bs = bias_slice(din["b2"], c * 512, 512)
                        nc.tensor.matmul(accs[tl][:], ones[0:1, 0:128], bs,
                                         start=True, stop=False)
                for ft in range(FT):
                    w2t = w2p.tile([128, 512], MD, tag="w2")
                    nc.sync.dma_start(
                        w2t[:],
                        din["w2"][ft * 128:(ft + 1) * 128,
                                  c * 512:(c + 1) * 512])
                    for tl in range(4):
                        lo = (ft % 2) * 512 + tl * 128
                        nc.tensor.matmul(
                            accs[tl][:],
                            gu_tiles[ft // 2][:, lo:lo + 128],
                            w2t[:],
                            start=(ft == 0 and not with_bias), stop=(ft == FT - 1))
                for tl in range(4):
                    t = hf * 4 + tl
                    ot = outp.tile([128, 512], F32, tag="outp")
                    nc.vector.tensor_add(ot[:],
                                         x2_tiles[t][:, c * 512:(c + 1) * 512],
                                         accs[tl][:])
                    nc.sync.dma_start(
                        d_out[t * 128:(t + 1) * 128, c * 512:(c + 1) * 512],
                        ot[:])


def _get_program(with_bias, mode=MM_MODE):
    key = ("prog", with_bias, mode)
    if key not in _CACHE:
        _CACHE[key] = _build_program(with_bias, mode)
    return _CACHE[key]


def _prepare(x, Wq, bq, Wk, bk, Wv, bv, Wo, bo, W1, b1, W2, b2,
             g1, be1, g2, be2, mode=MM_MODE):
    x = np.asarray(x, dtype=np.float32)
    f64 = np.float64
    mdt = _np_dt(mode)

    # Fold LN affine params into the following projections (exact algebra):
    # (z*g + be) @ W + b = z @ (g[:,None]*W) + (be @ W + b);
    # 1/sqrt(hd) folded into Wq/bq.
    scale_q = 1.0 / np.sqrt(np.float64(HD))
    wq_eff = (np.asarray(g1, f64)[:, None] * np.asarray(Wq, f64)) * scale_q
    bq_eff = (np.asarray(be1, f64) @ np.asarray(Wq, f64) + np.asarray(bq, f64)) * scale_q
    wk_eff = np.asarray(g1, f64)[:, None] * np.asarray(Wk, f64)
    bk_eff = np.asarray(be1, f64) @ np.asarray(Wk, f64) + np.asarray(bk, f64)
    wv_eff = np.asarray(g1, f64)[:, None] * np.asarray(Wv, f64)
    bv_eff = np.asarray(be1, f64) @ np.asarray(Wv, f64) + np.asarray(bv, f64)
    w1_eff = np.asarray(g2, f64)[:, None] * np.asarray(W1, f64)
    b1_eff = np.asarray(be2, f64) @ np.asarray(W1, f64) + np.asarray(b1, f64)

    biases = [bq_eff, bk_eff, bv_eff, np.asarray(bo, f64),
              b1_eff, np.asarray(b2, f64)]
    with_bias = any(np.any(b != 0.0) for b in biases)

    nc = _get_program(with_bias, mode)

    common = {
        "wq": np.ascontiguousarray(wq_eff.astype(mdt)),
        "wk": np.ascontiguousarray(wk_eff.astype(mdt)),
        "wv": np.ascontiguousarray(wv_eff.astype(mdt)),
        "wo": np.ascontiguousarray(np.asarray(Wo, f64).astype(mdt)),
        "w1": np.ascontiguousarray(w1_eff.astype(mdt)),
        "w2": np.ascontiguousarray(np.asarray(W2, f64).astype(mdt)),
        "bq": bq_eff.astype(mdt).reshape(1, D),
        "bk": bk_eff.astype(mdt).reshape(1, D),
        "bv": bv_eff.astype(mdt).reshape(1, D),
        "bo": np.asarray(bo, f64).astype(mdt).reshape(1, D),
        "b1": b1_eff.astype(mdt).reshape(1, FF),
        "b2": np.asarray(b2, f64).astype(mdt).reshape(1, D),
        "ident": np.eye(128, dtype=mdt if mode == "bf16" else np.float32),
        "ones": np.ones((1, 512), dtype=mdt),
        "onescol": np.ones((1, 16), dtype=mdt),
    }
    in_maps = []
    for b in range(NCORES):
        m = dict(common)
        m["x"] = np.ascontiguousarray(x[b])
        in_maps.append(m)
    return nc, in_maps


def kernel(**inputs):
    nc, in_maps = _prepare(**inputs)
    res = bass_utils.run_bass_kernel_spmd(nc, in_maps,
                                          core_ids=list(range(NCORES)))
    out = np.stack([res.results[b]["out"] for b in range(NCORES)], axis=0)
    return out.astype(np.float32)


def _timed_run(inputs, trace_cores=None):
    """Test-harness helper: rerun with NTFF tracing to get HW exec time."""
    nc, in_maps = _prepare(**inputs)
    try:
        return bass_utils.run_bass_kernel_spmd(
            nc, in_maps, core_ids=list(range(NCORES)), trace=True,
            trace_cores=trace_cores)
    except Exception as e:
        print(f"traced run failed: {e}")
        return None
